# revision 9
# baseline (speedup 1.0000x reference)
"""Trainium2 Bass kernel for nn_EstraNet_1443109012284.

Mathematical reduction: the reference's FAVOR+/trig branch (phi_q, aux_q/k,
fr_q/k, aux_A, A) does not feed the output.  The output is exactly

    out[b,n,d] = sum_{h,c} W_o[h,c,d] * norma[h] * sum_{d'} W_v[d',h,c] * x[b,n,d']
               = (x @ M)[b,n,d],   M[d',d] = sum_{h,c} W_v[d',h,c] norma[h] W_o[h,c,d]

with norma[h] = || sum_d s_p[h] W_p[d,h,:] beta_p[d] ||_2.

M is a tiny [512,512] matrix folded on the host; the device does the single
big GEMM  y[32768,512] = x[32768,512] @ M[512,512]  data-parallel over rows:
each of the 8 cores handles 4096 rows (yT[d, n] = sum_k M[k, d] xT[k, n]).

v2 schedule (per core), built from the v1 profile:
- 8 column-stripes of 512; per stripe 16 MMs (4 k-chunks x 4 d-blocks),
  k-outer so each of the 4 PSUM banks accumulates across the whole stripe
  and input-chunk deadlines are maximally late.  Two stripes in flight
  use all 8 PSUM banks; a stripe's banks drain (ACT copy -> fp16 tile ->
  DMA) while the next stripe computes.  Last stripe runs d-outer/k-inner
  so its banks retire staggered and the tail is short.
- The PE p-state ramp (~3.6us at half clock from the first matmul) is
  burned with a few warmup MMs on a memset tile, switching to real MMs
  as soon as the first m/x chunks land: real work proceeds at half rate
  during the remainder of the ramp instead of idling behind warmups.
- M is split into 4 k-chunks issued FIRST on each DMA queue (SP HW,
  ACT HW, Pool SW) so the first real matmul only waits for one 128KB
  chunk, not the whole 512KB of M.
- Input x is spread over all 3 queues in consumption order; first two
  stripes as 128KB chunks (early availability), rest as 256KB chunks
  (issue-cost amortization).  Outputs go to the SP HW queue (stripe 4 to
  the Pool SW queue to cover SP's input backlog) so the final stripes
  drain on an empty queue.
- fp16 end-to-end (x, M, y), M pre-scaled by a power of two on the host
  so values clear the fp16 subnormal range; the scale is divided back out
  on the host.
"""

import os as _os
import sys

sys.path.insert(0, "/opt/trn_rl_repo")

import numpy as np

import concourse.bass as bass
import concourse.tile as tile
from concourse import bacc, mybir
from concourse.bass_utils import run_bass_kernel_spmd

N_CORES = 8
ROWS = 32768           # B*N = 8*4096
RPC = ROWS // N_CORES  # rows per core = 4096
D = 512
KC = 4                 # contraction chunks of 128
DT = D // 128          # output row-blocks = 4
NS = 8                 # column stripes per core
SW = RPC // NS         # stripe width = 512

COMPUTE_DTYPE = _os.environ.get("KERNEL_DTYPE", "fp16")
N_WARM = int(_os.environ.get("KERNEL_NWARM", "7"))

_DT = {
    "fp32": mybir.dt.float32,
    "f32r": mybir.dt.float32r,
    "bf16": mybir.dt.bfloat16,
    "fp16": mybir.dt.float16,
}


def _np_dtype(token):
    if token == "bf16":
        import ml_dtypes

        return ml_dtypes.bfloat16
    if token == "fp16":
        return np.float16
    return np.float32


def _build(token):
    dt_in = _DT[token]
    dt_out = mybir.dt.float16 if token == "fp16" else mybir.dt.float32
    nc = bacc.Bacc("TRN2", target_bir_lowering=False)
    # x pre-transposed on the host.  Stripes 0-1 as [k, stripe] 128KB
    # chunks (early availability); stripes 2-4 / 5-7 as [k, 1536] 384KB
    # chunks (amortize the ~0.65us HWDGE / ~1.0us SWDGE issue cost).
    xs0 = nc.dram_tensor("xs0", [KC, 2, 128, SW], dt_in, kind="ExternalInput")
    xmid = nc.dram_tensor("xmid", [KC, 128, 3 * SW], dt_in, kind="ExternalInput")
    xend = nc.dram_tensor("xend", [KC, 128, 3 * SW], dt_in, kind="ExternalInput")
    mm = nc.dram_tensor("mm", [KC, 128, D], dt_in, kind="ExternalInput")
    yt = nc.dram_tensor("yt", [D, RPC], dt_out, kind="ExternalOutput")

    with tile.TileContext(nc) as tc:
        with (
            tc.tile_pool(name="wp", bufs=1) as wp,
            tc.tile_pool(name="mp", bufs=1) as mp,
            tc.tile_pool(name="xp", bufs=1) as xp,
            tc.tile_pool(name="op", bufs=1) as op,
            tc.tile_pool(name="pp", bufs=8, space="PSUM") as pp,
        ):
            # Warmup MMs start the PE p-state ramp immediately; they only
            # depend on a DVE memset (DVE is otherwise idle; gpsimd must
            # start issuing SW-queue DMAs right away).
            wz = wp.tile([128, 512], mybir.dt.bfloat16, name="wz")
            nc.vector.memset(wz[:], 1.0)
            warm = pp.tile([128, 512], mybir.dt.float32, tag="ps", name="warm")
            for _ in range(N_WARM):
                nc.tensor.matmul(warm[:], wz[:, 0:128], wz[:], start=True, stop=True)

            # --- input tiles ---
            m_sb = [mp.tile([128, D], dt_in, tag=f"m{k}", name=f"m{k}") for k in range(KC)]
            xt_s01 = {}
            for k in range(KC):
                for s in range(2):
                    xt_s01[(k, s)] = xp.tile([128, SW], dt_in, tag=f"a{k}{s}", name=f"a{k}{s}")
            xt_mid = {}
            xt_end = {}
            for k in range(KC):
                xt_mid[k] = xp.tile([128, 3 * SW], dt_in, tag=f"mid{k}", name=f"mid{k}")
                xt_end[k] = xp.tile([128, 3 * SW], dt_in, tag=f"end{k}", name=f"end{k}")

            # --- input DMA issue streams (order per engine == issue order) ---
            # All queues share ~360 GB/s of DMA-engine bandwidth round-robin,
            # so the critical first chunks (m0, k0s0) ride the SP queue while
            # the other queues' first doorbells are naturally delayed (ACT by
            # its table load, Pool by the const memsets + SWDGE issue cost).
            # SP HW queue
            nc.sync.dma_start(out=m_sb[0][:], in_=mm[0])
            nc.sync.dma_start(out=xt_s01[(0, 0)][:], in_=xs0[0, 0])
            nc.sync.dma_start(out=xt_s01[(1, 0)][:], in_=xs0[1, 0])
            nc.sync.dma_start(out=m_sb[1][:], in_=mm[1])
            nc.sync.dma_start(out=xt_s01[(1, 1)][:], in_=xs0[1, 1])
            nc.sync.dma_start(out=xt_mid[0][:], in_=xmid[0])
            nc.sync.dma_start(out=xt_mid[3][:], in_=xmid[3])
            nc.sync.dma_start(out=xt_end[0][:], in_=xend[0])
            nc.sync.dma_start(out=xt_end[3][:], in_=xend[3])
            # ACT HW queue (first doorbell delayed by the ACT table load;
            # issues finish before the first PSUM copy is needed)
            nc.scalar.dma_start(out=xt_s01[(2, 0)][:], in_=xs0[2, 0])
            nc.scalar.dma_start(out=xt_s01[(0, 1)][:], in_=xs0[0, 1])
            nc.scalar.dma_start(out=xt_s01[(2, 1)][:], in_=xs0[2, 1])
            nc.scalar.dma_start(out=xt_mid[2][:], in_=xmid[2])
            nc.scalar.dma_start(out=xt_end[2][:], in_=xend[2])
            # Pool SW queue
            nc.gpsimd.dma_start(out=m_sb[2][:], in_=mm[2])
            nc.gpsimd.dma_start(out=m_sb[3][:], in_=mm[3])
            nc.gpsimd.dma_start(out=xt_s01[(3, 0)][:], in_=xs0[3, 0])
            nc.gpsimd.dma_start(out=xt_s01[(3, 1)][:], in_=xs0[3, 1])
            nc.gpsimd.dma_start(out=xt_mid[1][:], in_=xmid[1])
            nc.gpsimd.dma_start(out=xt_end[1][:], in_=xend[1])

            def xslice(k, s):
                if s < 2:
                    return xt_s01[(k, s)][:]
                if s < 5:
                    return xt_mid[k][:, (s - 2) * SW : (s - 1) * SW]
                return xt_end[k][:, (s - 5) * SW : (s - 4) * SW]

            # Output: stripe pairs (0,1),(2,3),(4,5) -> [128,1024] tiles
            # (one 256KB DMA for two stripes), stripes 6,7 -> [128,512]
            # singles for a tight drain.  DMA engine per tile chosen so the
            # backlog drains on whichever queue is free and the final
            # stripes land on the empty SP queue.
            pair_tiles = {}

            for s in range(NS):
                pss = [
                    pp.tile([128, SW], mybir.dt.float32, tag="ps", name=f"ps_{s}_{d}")
                    for d in range(DT)
                ]
                if s < NS - 2:
                    # k-outer: latest possible input deadlines
                    for k in range(KC):
                        for d in range(DT):
                            nc.tensor.matmul(
                                pss[d][:],
                                m_sb[k][:, d * 128 : (d + 1) * 128],
                                xslice(k, s),
                                start=(k == 0),
                                stop=(k == KC - 1),
                            )
                else:
                    # last stripes d-outer: banks retire staggered -> short tail
                    for d in range(DT):
                        for k in range(KC):
                            nc.tensor.matmul(
                                pss[d][:],
                                m_sb[k][:, d * 128 : (d + 1) * 128],
                                xslice(k, s),
                                start=(k == 0),
                                stop=(k == KC - 1),
                            )
                if s < 6:
                    pair, par = divmod(s, 2)
                    if par == 0:
                        pair_tiles[pair] = [
                            op.tile([128, 2 * SW], dt_out, tag=f"pt{pair}{d}", name=f"pt{pair}_{d}")
                            for d in range(DT)
                        ]
                    for d in range(DT):
                        nc.scalar.copy(
                            pair_tiles[pair][d][:, par * SW : (par + 1) * SW], pss[d][:]
                        )
                    if par == 1:
                        # pair 0 -> Pool; pair 1 -> SP; pair 2 -> SP/Pool split
                        for d in range(DT):
                            if pair == 0:
                                oeng = nc.gpsimd
                            elif pair == 1:
                                oeng = nc.sync
                            else:
                                oeng = nc.sync if d < 2 else nc.gpsimd
                            oeng.dma_start(
                                out=yt[d * 128 : (d + 1) * 128, pair * 2 * SW : (pair + 1) * 2 * SW],
                                in_=pair_tiles[pair][d][:],
                            )
                else:
                    # stripes 6,7 retire d-staggered; stripe 6 splits across
                    # SP/Pool, stripe 7 drains entirely on SP (empty by then)
                    for d in range(DT):
                        ot = op.tile([128, SW], dt_out, tag=f"ot{s}{d}", name=f"ot{s}_{d}")
                        nc.scalar.copy(ot[:], pss[d][:])
                        if s == 6:
                            oeng = nc.sync if d < 2 else nc.gpsimd
                        else:
                            oeng = nc.sync
                        oeng.dma_start(
                            out=yt[d * 128 : (d + 1) * 128, s * SW : (s + 1) * SW],
                            in_=ot[:],
                        )
    nc.compile()
    return nc


def _fold_m(W_v, s_p, W_p, beta_p, W_o):
    """Host-side constant folding of the tiny parameter tensors into M."""
    W_v = np.asarray(W_v, dtype=np.float64)
    s_p = np.asarray(s_p, dtype=np.float64)
    W_p = np.asarray(W_p, dtype=np.float64)
    beta_p = np.asarray(beta_p, dtype=np.float64)
    W_o = np.asarray(W_o, dtype=np.float64)
    phi = np.einsum("h,dhc,d->hc", s_p, W_p, beta_p)
    norma = np.linalg.norm(phi, axis=1)  # [h]
    M = np.einsum("dhc,h,hce->de", W_v, norma, W_o)  # [512, 512]
    return M.astype(np.float32)


_prog_cache = {}
_last_in_maps = None  # kept for test.py profiling reuse
_last_result = None


def _run(in_maps, token, **kwargs):
    if token not in _prog_cache:
        _prog_cache[token] = _build(token)
    return run_bass_kernel_spmd(_prog_cache[token], in_maps, list(range(N_CORES)), **kwargs)


def kernel(x, W_v, s_p, c_p, W_p, W_A, W_o, beta_p, beta_i_p, **_unused):
    global _last_in_maps, _last_result
    token = COMPUTE_DTYPE
    np_dt = _np_dtype(token)

    x = np.asarray(x, dtype=np.float32)
    M = _fold_m(W_v, s_p, W_p, beta_p, W_o)

    # fp16 path: scale M by an exact power of two so M entries and y values
    # sit in fp16 normal range; undo on the host after the run
    out_unscale = 1.0
    if token == "fp16":
        amax = float(np.abs(M).max())
        if amax > 0:
            e = int(np.floor(-np.log2(amax)))
            M = M * np.float32(2.0**e)
            out_unscale = 2.0**-e

    B, N, Dd = x.shape
    assert B * N == ROWS and Dd == D, (x.shape,)

    mmc = np.ascontiguousarray(M.reshape(KC, 128, D)).astype(np_dt)
    xf = x.reshape(ROWS, D)

    in_maps = []
    for c in range(N_CORES):
        sh = xf[c * RPC : (c + 1) * RPC]               # [4096, 512]
        xT = sh.T.astype(np_dt)                        # [512, 4096]
        xk = xT.reshape(KC, 128, NS, SW)               # [k, part, stripe, col]
        xs0 = np.ascontiguousarray(xk[:, :, 0:2].transpose(0, 2, 1, 3))
        xmid = np.ascontiguousarray(xk[:, :, 2:5].reshape(KC, 128, 3 * SW))
        xend = np.ascontiguousarray(xk[:, :, 5:8].reshape(KC, 128, 3 * SW))
        in_maps.append({"xs0": xs0, "xmid": xmid, "xend": xend, "mm": mmc})

    _last_in_maps = in_maps
    res = _run(in_maps, token)
    _last_result = res
    out = np.empty((ROWS, D), dtype=np.float32)
    for c in range(N_CORES):
        yc = res.results[c]["yt"].astype(np.float32)
        if out_unscale != 1.0:
            yc *= np.float32(out_unscale)
        out[c * RPC : (c + 1) * RPC] = yc.T
    return out.reshape(B, N, D)


if __name__ == "__main__":
    # smoke test with random data
    rng = np.random.default_rng(0)
    x = rng.standard_normal((8, 4096, 512)).astype(np.float32)
    W_v = rng.standard_normal((512, 8, 64)).astype(np.float32) * 0.01
    s_p = np.ones((8,), np.float32)
    c_p = np.ones((8,), np.float32)
    W_p = rng.standard_normal((512, 8, 64)).astype(np.float32) * 0.01
    W_A = rng.standard_normal((256, 64)).astype(np.float32)
    W_o = rng.standard_normal((8, 64, 512)).astype(np.float32) * 0.01
    beta_p = rng.standard_normal((512,)).astype(np.float32) * 1e-5
    beta_i_p = rng.standard_normal((4096, 512)).astype(np.float32) * 1e-5
    out = kernel(x, W_v=W_v, s_p=s_p, c_p=c_p, W_p=W_p, W_A=W_A, W_o=W_o,
                 beta_p=beta_p, beta_i_p=beta_i_p)
    M = _fold_m(W_v, s_p, W_p, beta_p, W_o)
    exp = (x.reshape(-1, 512).astype(np.float64) @ M.astype(np.float64)).reshape(8, 4096, 512)
    err = np.abs(out - exp).max() / (np.abs(exp).max() + 1e-30)
    print("smoke rel err:", err)


# revision 12
# speedup vs baseline: 1.0809x; 1.0809x over previous
"""Trainium2 Bass kernel for nn_EstraNet_1443109012284.

Mathematical reduction: the reference's FAVOR+/trig branch (phi_q, aux_q/k,
fr_q/k, aux_A, A) does not feed the output.  The output is exactly

    out[b,n,d] = sum_{h,c} W_o[h,c,d] * norma[h] * sum_{d'} W_v[d',h,c] * x[b,n,d']
               = (x @ M)[b,n,d],   M[d',d] = sum_{h,c} W_v[d',h,c] norma[h] W_o[h,c,d]

with norma[h] = || sum_d s_p[h] W_p[d,h,:] beta_p[d] ||_2.

M is a tiny [512,512] matrix folded on the host; the device does the single
big GEMM  y[32768,512] = x[32768,512] @ M[512,512]  data-parallel over rows:
each of the 8 cores handles 4096 rows.

Device design (per core): compute yT[d, n] = sum_k M[k, d] * xT[k, n]
- lhsT (stationary) = M chunk [128k x 128d]; rhs (moving) = xT quarter
  [128k x 512n], fed pre-transposed from the host (no on-device transpose).
- Same/alternating-weight MMs pipeline at 512/2.4GHz = 216 ns.
- PSUM->SBUF copies all on ONE engine (ACT): PE drain + a single reader
  share PSUM fine; two concurrent readers throttle the PE ~2.3x.
- PE warmed up with dummy matmuls (dep: a memset tile only) during the
  input-DMA window so the HAM clock ramp doesn't tax real work.
- fp16 path (default): x, M, y all fp16, M pre-scaled by an exact power of
  two so M / y avoid the fp16 subnormal range; host multiplies the scale
  back out.  fp16 keeps 10 mantissa bits (vs bf16's 7) and halves output
  DMA vs fp32 -> kernel is PE-bound at ~216ns per [128x128]x[128x512] MM.
"""

import os as _os
import sys

sys.path.insert(0, "/opt/trn_rl_repo")

import numpy as np

import concourse.bass as bass
import concourse.tile as tile
from concourse import bacc, mybir
from concourse.bass_utils import run_bass_kernel_spmd

N_CORES = 8
ROWS = 32768           # B*N = 8*4096
RPC = ROWS // N_CORES  # rows per core = 4096
D = 512
KC = 4                 # contraction chunks of 128
DT = D // 128          # output row-blocks = 4
HB = 4                 # n-quarters per stripe
HW = RPC // HB         # 1024 columns per quarter
JH = HW // 512         # moving chunks of 512 per phase = 2

COMPUTE_DTYPE = _os.environ.get("KERNEL_DTYPE", "fp16")
N_WARM = int(_os.environ.get("KERNEL_NWARM", "8"))

_DT = {
    "fp32": mybir.dt.float32,
    "f32r": mybir.dt.float32r,
    "bf16": mybir.dt.bfloat16,
    "fp16": mybir.dt.float16,
}


def _np_dtype(token):
    if token == "bf16":
        import ml_dtypes

        return ml_dtypes.bfloat16
    if token == "fp16":
        return np.float16
    return np.float32


def _build(token):
    dt_in = _DT[token]
    dt_out = mybir.dt.float16 if token == "fp16" else mybir.dt.float32
    nc = bacc.Bacc("TRN2", target_bir_lowering=False)
    # x pre-transposed, [k-chunk, quarter, 128, 1024]: each quarter-stripe is
    # one contiguous DMA
    xt = nc.dram_tensor("xt", [KC, HB, 128, HW], dt_in, kind="ExternalInput")
    mm = nc.dram_tensor("mm", [128, KC, D], dt_in, kind="ExternalInput")
    yt = nc.dram_tensor("yt", [D, RPC], dt_out, kind="ExternalOutput")

    with tile.TileContext(nc) as tc:
        with (
            tc.tile_pool(name="xp", bufs=1) as xp,
            tc.tile_pool(name="mp", bufs=1) as mp,
            tc.tile_pool(name="op", bufs=4) as op,
            tc.tile_pool(name="pp", bufs=8, space="PSUM") as pp,
        ):
            # PE warmup: matmuls that depend only on a memset tile start at
            # ~6us (right after engine code load) and burn the HAM
            # cold-clock ramp while the x DMAs are still in flight.
            # Always bf16: warmup dtype is independent of the compute dtype
            # (and memset doesn't support float32r).
            wz = mp.tile([128, 512], mybir.dt.bfloat16, name="wz")
            nc.gpsimd.memset(wz[:], 1.0)
            warm = pp.tile([128, 512], mybir.dt.float32, tag="ps", name="warm")
            for w in range(N_WARM):
                nc.tensor.matmul(
                    warm[:], wz[:, 0:128], wz[:], start=True, stop=True
                )

            # m split in halves, one FIRST on each HWDGE queue: the first
            # real matmul only waits for a 256KB chunk instead of the whole
            # 512KB of m serializing one queue ahead of the x stream
            m_sb = mp.tile([128, KC, D], dt_in, name="m_sb")
            nc.sync.dma_start(out=m_sb[:, 0:2], in_=mm[:, 0:2])
            nc.scalar.dma_start(out=m_sb[:, 2:4], in_=mm[:, 2:4])

            x_sb = {}
            for h in range(HB):
                for k in range(KC):
                    t = xp.tile([128, HW], dt_in, tag=f"x{k}{h}", name=f"x{k}{h}")
                    eng = nc.sync if (h * KC + k) % 2 == 0 else nc.scalar
                    eng.dma_start(out=t[:], in_=xt[k, h])
                    x_sb[(k, h)] = t

            # phases: h outer (first phase only needs the first 4 quarter
            # DMAs), d inner.  k-major MM order (4 weight switches per
            # phase, banks finish staggered); last phase j-major with per-
            # bank copy+DMA so the tail is short.
            NPH = HB * DT
            for ph in range(NPH):
                h, d = divmod(ph, DT)
                d0 = d * 128
                last = ph == NPH - 1
                ot = op.tile([128, HW], dt_out, name=f"ot{ph}", tag="ot")
                pss = [
                    pp.tile([128, 512], mybir.dt.float32, tag="ps", name=f"ps_{h}_{d}_{j}")
                    for j in range(JH)
                ]
                # output DMAs rotate gpsimd/sync/scalar: Pool's SW ring and
                # both HWDGE queues share the load so no single queue backs
                # up into the kernel tail.  Scalar only takes every 4th
                # phase (its sequencer also runs the PSUM copies; one
                # 0.65us DMA issue fits in the ~0.8us copy slack of two
                # phases).
                if ph % 2 == 0:
                    oeng = nc.gpsimd
                elif ph % 4 == 3:
                    oeng = nc.scalar
                else:
                    oeng = nc.sync
                if last:
                    for j in range(JH):
                        for k in range(KC):
                            nc.tensor.matmul(
                                pss[j][:],
                                m_sb[:, k, d0 : d0 + 128],
                                x_sb[(k, h)][:, j * 512 : (j + 1) * 512],
                                start=(k == 0),
                                stop=(k == KC - 1),
                            )
                        # final phase: half-granularity copies + stores on the
                        # warm sync HWDGE ring so the last transfer overlaps
                        # the last copy (SWDGE's slow first-byte would pad the
                        # kernel tail)
                        for q in range(2):
                            c0 = j * 512 + q * 256
                            nc.scalar.copy(ot[:, c0 : c0 + 256], pss[j][:, q * 256 : (q + 1) * 256])
                            nc.sync.dma_start(
                                out=yt[d0 : d0 + 128, h * HW + c0 : h * HW + c0 + 256],
                                in_=ot[:, c0 : c0 + 256],
                            )
                else:
                    for k in range(KC):
                        for j in range(JH):
                            nc.tensor.matmul(
                                pss[j][:],
                                m_sb[:, k, d0 : d0 + 128],
                                x_sb[(k, h)][:, j * 512 : (j + 1) * 512],
                                start=(k == 0),
                                stop=(k == KC - 1),
                            )
                    for j in range(JH):
                        nc.scalar.copy(ot[:, j * 512 : (j + 1) * 512], pss[j][:])
                    oeng.dma_start(
                        out=yt[d0 : d0 + 128, h * HW : (h + 1) * HW], in_=ot[:]
                    )
    nc.compile()
    return nc


def _fold_m(W_v, s_p, W_p, beta_p, W_o):
    """Host-side constant folding of the tiny parameter tensors into M."""
    W_v = np.asarray(W_v, dtype=np.float64)
    s_p = np.asarray(s_p, dtype=np.float64)
    W_p = np.asarray(W_p, dtype=np.float64)
    beta_p = np.asarray(beta_p, dtype=np.float64)
    W_o = np.asarray(W_o, dtype=np.float64)
    phi = np.einsum("h,dhc,d->hc", s_p, W_p, beta_p)
    norma = np.linalg.norm(phi, axis=1)  # [h]
    M = np.einsum("dhc,h,hce->de", W_v, norma, W_o)  # [512, 512]
    return M.astype(np.float32)


_prog_cache = {}
_last_in_maps = None  # kept for test.py profiling reuse
_last_result = None


def _run(in_maps, token, **kwargs):
    if token not in _prog_cache:
        _prog_cache[token] = _build(token)
    return run_bass_kernel_spmd(_prog_cache[token], in_maps, list(range(N_CORES)), **kwargs)


def kernel(x, W_v, s_p, c_p, W_p, W_A, W_o, beta_p, beta_i_p, **_unused):
    global _last_in_maps, _last_result
    token = COMPUTE_DTYPE
    np_dt = _np_dtype(token)

    x = np.asarray(x, dtype=np.float32)
    M = _fold_m(W_v, s_p, W_p, beta_p, W_o)

    # fp16 path: scale M by an exact power of two so M entries and y values
    # sit in fp16 normal range; undo on the host after the run
    out_unscale = 1.0
    if token == "fp16":
        amax = float(np.abs(M).max())
        if amax > 0:
            e = int(np.floor(-np.log2(amax)))
            M = M * np.float32(2.0**e)
            out_unscale = 2.0**-e

    B, N, Dd = x.shape
    assert B * N == ROWS and Dd == D, (x.shape,)

    mmc = np.ascontiguousarray(M.reshape(KC, 128, D).transpose(1, 0, 2)).astype(np_dt)
    xf = x.reshape(ROWS, D)

    in_maps = []
    for c in range(N_CORES):
        sh = xf[c * RPC : (c + 1) * RPC]               # [4096, 512]
        xT = sh.T.astype(np_dt)                        # [512, 4096]
        # [KC, 128, HB, HW] -> [KC, HB, 128, HW], each quarter contiguous
        xs = np.ascontiguousarray(
            xT.reshape(KC, 128, HB, HW).transpose(0, 2, 1, 3)
        )
        in_maps.append({"xt": xs, "mm": mmc})

    _last_in_maps = in_maps
    res = _run(in_maps, token)
    _last_result = res
    out = np.empty((ROWS, D), dtype=np.float32)
    for c in range(N_CORES):
        yc = res.results[c]["yt"].astype(np.float32)
        if out_unscale != 1.0:
            yc *= np.float32(out_unscale)
        out[c * RPC : (c + 1) * RPC] = yc.T
    return out.reshape(B, N, D)


if __name__ == "__main__":
    # smoke test with random data
    rng = np.random.default_rng(0)
    x = rng.standard_normal((8, 4096, 512)).astype(np.float32)
    W_v = rng.standard_normal((512, 8, 64)).astype(np.float32) * 0.01
    s_p = np.ones((8,), np.float32)
    c_p = np.ones((8,), np.float32)
    W_p = rng.standard_normal((512, 8, 64)).astype(np.float32) * 0.01
    W_A = rng.standard_normal((256, 64)).astype(np.float32)
    W_o = rng.standard_normal((8, 64, 512)).astype(np.float32) * 0.01
    beta_p = rng.standard_normal((512,)).astype(np.float32) * 1e-5
    beta_i_p = rng.standard_normal((4096, 512)).astype(np.float32) * 1e-5
    out = kernel(x, W_v=W_v, s_p=s_p, c_p=c_p, W_p=W_p, W_A=W_A, W_o=W_o,
                 beta_p=beta_p, beta_i_p=beta_i_p)
    M = _fold_m(W_v, s_p, W_p, beta_p, W_o)
    exp = (x.reshape(-1, 512).astype(np.float64) @ M.astype(np.float64)).reshape(8, 4096, 512)
    err = np.abs(out - exp).max() / (np.abs(exp).max() + 1e-30)
    print("smoke rel err:", err)



# revision 14
# speedup vs baseline: 1.1169x; 1.0333x over previous
"""Trainium2 Bass kernel for nn_EstraNet_1443109012284.

Mathematical reduction: the reference's FAVOR+/trig branch (phi_q, aux_q/k,
fr_q/k, aux_A, A) does not feed the output.  The output is exactly

    out[b,n,d] = sum_{h,c} W_o[h,c,d] * norma[h] * sum_{d'} W_v[d',h,c] * x[b,n,d']
               = (x @ M)[b,n,d],   M[d',d] = sum_{h,c} W_v[d',h,c] norma[h] W_o[h,c,d]

with norma[h] = || sum_d s_p[h] W_p[d,h,:] beta_p[d] ||_2.

M is a tiny [512,512] matrix folded on the host; the device does the single
big GEMM  y[32768,512] = x[32768,512] @ M[512,512]  data-parallel over rows:
each of the 8 cores handles 4096 rows.

Device design (per core): compute yT[d, n] = sum_k M[k, d] * xT[k, n]
- lhsT (stationary) = M chunk [128k x 128d]; rhs (moving) = xT quarter
  [128k x 512n], fed pre-transposed from the host (no on-device transpose).
- Same/alternating-weight MMs pipeline at 512/2.4GHz = 216 ns.
- PSUM->SBUF copies all on ONE engine (ACT): PE drain + a single reader
  share PSUM fine; two concurrent readers throttle the PE ~2.3x.
- PE warmed up with dummy matmuls (dep: a memset tile only) during the
  input-DMA window so the HAM clock ramp doesn't tax real work.
- fp16 path (default): x, M, y all fp16, M pre-scaled by an exact power of
  two so M / y avoid the fp16 subnormal range; host multiplies the scale
  back out.  fp16 keeps 10 mantissa bits (vs bf16's 7) and halves output
  DMA vs fp32 -> kernel is PE-bound at ~216ns per [128x128]x[128x512] MM.
"""

import os as _os
import sys

sys.path.insert(0, "/opt/trn_rl_repo")

import numpy as np

import concourse.bass as bass
import concourse.tile as tile
from concourse import bacc, mybir
from concourse.bass_utils import run_bass_kernel_spmd

N_CORES = 8
ROWS = 32768           # B*N = 8*4096
RPC = ROWS // N_CORES  # rows per core = 4096
D = 512
KC = 4                 # contraction chunks of 128
DT = D // 128          # output row-blocks = 4
HB = 4                 # n-quarters per stripe
HW = RPC // HB         # 1024 columns per quarter
JH = HW // 512         # moving chunks of 512 per phase = 2

COMPUTE_DTYPE = _os.environ.get("KERNEL_DTYPE", "fp16")
N_WARM = int(_os.environ.get("KERNEL_NWARM", "8"))

_DT = {
    "fp32": mybir.dt.float32,
    "f32r": mybir.dt.float32r,
    "bf16": mybir.dt.bfloat16,
    "fp16": mybir.dt.float16,
}


def _np_dtype(token):
    if token == "bf16":
        import ml_dtypes

        return ml_dtypes.bfloat16
    if token == "fp16":
        return np.float16
    return np.float32


def _build(token):
    dt_in = _DT[token]
    dt_out = mybir.dt.float16 if token == "fp16" else mybir.dt.float32
    nc = bacc.Bacc("TRN2", target_bir_lowering=False)
    # x pre-transposed, [k-chunk, quarter, 128, 1024]: each quarter-stripe is
    # one contiguous DMA
    xt = nc.dram_tensor("xt", [KC, HB, 128, HW], dt_in, kind="ExternalInput")
    mm = nc.dram_tensor("mm", [128, KC, D], dt_in, kind="ExternalInput")
    yt = nc.dram_tensor("yt", [D, RPC], dt_out, kind="ExternalOutput")

    with tile.TileContext(nc) as tc:
        with (
            tc.tile_pool(name="xp", bufs=1) as xp,
            tc.tile_pool(name="mp", bufs=1) as mp,
            tc.tile_pool(name="op", bufs=4) as op,
            tc.tile_pool(name="pp", bufs=8, space="PSUM") as pp,
        ):
            # PE warmup: matmuls that depend only on a memset tile start at
            # ~6us (right after engine code load) and burn the HAM
            # cold-clock ramp while the x DMAs are still in flight.
            # Always bf16: warmup dtype is independent of the compute dtype
            # (and memset doesn't support float32r).
            wz = mp.tile([128, 512], mybir.dt.bfloat16, name="wz")
            nc.gpsimd.memset(wz[:], 1.0)
            warm = pp.tile([128, 512], mybir.dt.float32, tag="ps", name="warm")
            for w in range(N_WARM):
                nc.tensor.matmul(
                    warm[:], wz[:, 0:128], wz[:], start=True, stop=True
                )

            # m split in halves, one FIRST on each HWDGE queue: the first
            # real matmul only waits for a 256KB chunk instead of the whole
            # 512KB of m serializing one queue ahead of the x stream
            m_sb = mp.tile([128, KC, D], dt_in, name="m_sb")
            nc.sync.dma_start(out=m_sb[:, 0:2], in_=mm[:, 0:2])
            nc.scalar.dma_start(out=m_sb[:, 2:4], in_=mm[:, 2:4])

            x_sb = {}
            for h in range(HB):
                for k in range(KC):
                    t = xp.tile([128, HW], dt_in, tag=f"x{k}{h}", name=f"x{k}{h}")
                    eng = nc.sync if (h * KC + k) % 2 == 0 else nc.scalar
                    eng.dma_start(out=t[:], in_=xt[k, h])
                    x_sb[(k, h)] = t

            # phases: h outer (first phase only needs the first 4 quarter
            # DMAs), d inner.  k-major MM order (4 weight switches per
            # phase, banks finish staggered); last phase j-major with per-
            # bank copy+DMA so the tail is short.
            NPH = HB * DT
            for ph in range(NPH):
                h, d = divmod(ph, DT)
                d0 = d * 128
                last = ph == NPH - 1
                ot = op.tile([128, HW], dt_out, name=f"ot{ph}", tag="ot")
                pss = [
                    pp.tile([128, 512], mybir.dt.float32, tag="ps", name=f"ps_{h}_{d}_{j}")
                    for j in range(JH)
                ]
                # alternate output DMAs between the sync HWDGE queue and the
                # gpsimd SWDGE rings (POOL sequencer is otherwise idle) so
                # input and output streams don't serialize on one ring.
                # Scalar's sequencer is copy-only: a DMA issue between copies
                # delays the PSUM drain and back-pressures the PE.
                oeng = nc.gpsimd if ph % 2 == 0 else nc.sync
                if last:
                    # final phase: quarter-granularity copies, each quarter's
                    # store on a DIFFERENT engine's queue (sync x2, gpsimd,
                    # scalar-as-its-last-instruction) so the four issues
                    # don't serialize on one sequencer after the last MM
                    q_eng = [nc.sync, nc.gpsimd, nc.sync, None]
                    tail_dma = []
                    for j in range(JH):
                        for k in range(KC):
                            nc.tensor.matmul(
                                pss[j][:],
                                m_sb[:, k, d0 : d0 + 128],
                                x_sb[(k, h)][:, j * 512 : (j + 1) * 512],
                                start=(k == 0),
                                stop=(k == KC - 1),
                            )
                        for q in range(2):
                            c0 = j * 512 + q * 256
                            nc.scalar.copy(ot[:, c0 : c0 + 256], pss[j][:, q * 256 : (q + 1) * 256])
                            eng = q_eng[j * 2 + q]
                            args = dict(
                                out=yt[d0 : d0 + 128, h * HW + c0 : h * HW + c0 + 256],
                                in_=ot[:, c0 : c0 + 256],
                            )
                            if eng is None:
                                tail_dma.append(args)  # issue on ACT after all copies
                            else:
                                eng.dma_start(**args)
                    for args in tail_dma:
                        nc.scalar.dma_start(**args)
                else:
                    for k in range(KC):
                        for j in range(JH):
                            nc.tensor.matmul(
                                pss[j][:],
                                m_sb[:, k, d0 : d0 + 128],
                                x_sb[(k, h)][:, j * 512 : (j + 1) * 512],
                                start=(k == 0),
                                stop=(k == KC - 1),
                            )
                    for j in range(JH):
                        nc.scalar.copy(ot[:, j * 512 : (j + 1) * 512], pss[j][:])
                    oeng.dma_start(
                        out=yt[d0 : d0 + 128, h * HW : (h + 1) * HW], in_=ot[:]
                    )
    nc.compile()
    return nc


def _fold_m(W_v, s_p, W_p, beta_p, W_o):
    """Host-side constant folding of the tiny parameter tensors into M."""
    W_v = np.asarray(W_v, dtype=np.float64)
    s_p = np.asarray(s_p, dtype=np.float64)
    W_p = np.asarray(W_p, dtype=np.float64)
    beta_p = np.asarray(beta_p, dtype=np.float64)
    W_o = np.asarray(W_o, dtype=np.float64)
    phi = np.einsum("h,dhc,d->hc", s_p, W_p, beta_p)
    norma = np.linalg.norm(phi, axis=1)  # [h]
    M = np.einsum("dhc,h,hce->de", W_v, norma, W_o)  # [512, 512]
    return M.astype(np.float32)


_prog_cache = {}
_last_in_maps = None  # kept for test.py profiling reuse
_last_result = None


def _run(in_maps, token, **kwargs):
    if token not in _prog_cache:
        _prog_cache[token] = _build(token)
    return run_bass_kernel_spmd(_prog_cache[token], in_maps, list(range(N_CORES)), **kwargs)


def kernel(x, W_v, s_p, c_p, W_p, W_A, W_o, beta_p, beta_i_p, **_unused):
    global _last_in_maps, _last_result
    token = COMPUTE_DTYPE
    np_dt = _np_dtype(token)

    x = np.asarray(x, dtype=np.float32)
    M = _fold_m(W_v, s_p, W_p, beta_p, W_o)

    # fp16 path: scale M by an exact power of two so M entries and y values
    # sit in fp16 normal range; undo on the host after the run
    out_unscale = 1.0
    if token == "fp16":
        amax = float(np.abs(M).max())
        if amax > 0:
            e = int(np.floor(-np.log2(amax)))
            M = M * np.float32(2.0**e)
            out_unscale = 2.0**-e

    B, N, Dd = x.shape
    assert B * N == ROWS and Dd == D, (x.shape,)

    mmc = np.ascontiguousarray(M.reshape(KC, 128, D).transpose(1, 0, 2)).astype(np_dt)
    xf = x.reshape(ROWS, D)

    in_maps = []
    for c in range(N_CORES):
        sh = xf[c * RPC : (c + 1) * RPC]               # [4096, 512]
        xT = sh.T.astype(np_dt)                        # [512, 4096]
        # [KC, 128, HB, HW] -> [KC, HB, 128, HW], each quarter contiguous
        xs = np.ascontiguousarray(
            xT.reshape(KC, 128, HB, HW).transpose(0, 2, 1, 3)
        )
        in_maps.append({"xt": xs, "mm": mmc})

    _last_in_maps = in_maps
    res = _run(in_maps, token)
    _last_result = res
    out = np.empty((ROWS, D), dtype=np.float32)
    for c in range(N_CORES):
        yc = res.results[c]["yt"].astype(np.float32)
        if out_unscale != 1.0:
            yc *= np.float32(out_unscale)
        out[c * RPC : (c + 1) * RPC] = yc.T
    return out.reshape(B, N, D)


if __name__ == "__main__":
    # smoke test with random data
    rng = np.random.default_rng(0)
    x = rng.standard_normal((8, 4096, 512)).astype(np.float32)
    W_v = rng.standard_normal((512, 8, 64)).astype(np.float32) * 0.01
    s_p = np.ones((8,), np.float32)
    c_p = np.ones((8,), np.float32)
    W_p = rng.standard_normal((512, 8, 64)).astype(np.float32) * 0.01
    W_A = rng.standard_normal((256, 64)).astype(np.float32)
    W_o = rng.standard_normal((8, 64, 512)).astype(np.float32) * 0.01
    beta_p = rng.standard_normal((512,)).astype(np.float32) * 1e-5
    beta_i_p = rng.standard_normal((4096, 512)).astype(np.float32) * 1e-5
    out = kernel(x, W_v=W_v, s_p=s_p, c_p=c_p, W_p=W_p, W_A=W_A, W_o=W_o,
                 beta_p=beta_p, beta_i_p=beta_i_p)
    M = _fold_m(W_v, s_p, W_p, beta_p, W_o)
    exp = (x.reshape(-1, 512).astype(np.float64) @ M.astype(np.float64)).reshape(8, 4096, 512)
    err = np.abs(out - exp).max() / (np.abs(exp).max() + 1e-30)
    print("smoke rel err:", err)

